# revision 1
# baseline (speedup 1.0000x reference)
"""Trainium2 Bass kernel for nn_DemandPredictionModel (2-layer GCN + time MLP).

Strategy (8 NeuronCores, SPMD single program):
  - Shard nodes: core c owns rows [c*6250, (c+1)*6250).
  - Layer k: each core computes xws = dinv * (x @ Wk) for its rows (fp32
    matmul via PE-transpose), casts to bf16, AllGather -> full [50000,256]
    bf16 table in local HBM.
  - Edge aggregation: edges with dst in the core's range are gathered by
    src row via dma_gather (int16 idx; src split in lo/hi buckets at 25000
    to fit the int16 range), then scatter-added into PSUM with one
    [128edges x 128dst] 0/1 indicator matmul per 128-edge tile (fp8
    indicators, precomputed host-side).  GCN norm deg^-1/2 A deg^-1/2 is
    factored as dinv[src] (folded into the gathered table) times dinv[dst]
    (applied in the activation epilogue, scale AP).  Per-channel bias b is
    added inside PSUM via a rank-1 matmul sqrt(deg) (x) b so that
    dinv*(psum + sqrtdeg (x) b) = dinv*agg + b exactly.
  - Self-loop edges are gathered from the core's own shard (pre-AllGather).
  - Head: x_time = relu(tf @ Wt + bt) via a K=2 matmul; final projection
    out = [x_gcn, x_time] @ Wf + bf via DVE multiply + free-dim reduce.
"""

import sys

if "/opt/trn_rl_repo" not in sys.path:
    sys.path.insert(0, "/opt/trn_rl_repo")

import numpy as np
import ml_dtypes

import concourse.bacc as bacc
import concourse.bass as bass
import concourse.mybir as mybir
import concourse.tile as tile
from concourse import library_config
from concourse.bass_utils import run_bass_kernel_spmd
from concourse._compat import cdiv

N_NODES = 50000
N_EDGES = 800000
CH = 256
NCORES = 8
RPC = N_NODES // NCORES            # 6250 rows per core
NBLK = cdiv(RPC, 128)              # 49 blocks per core (last has 106 rows)
RPC_PAD = NBLK * 128               # 6272
SPLIT = 25000                      # lo: src < SPLIT, hi: src >= SPLIT
CHUNK_TILES = 8                    # gather chunk size (x128 edges); dma_gather
                                   # is capped at ~1024 idxs by the SWDGE ring
NQUEUES = 4                        # cycle gathers over all 4 SWDGE queues

F8 = ml_dtypes.float8_e4m3
BF16 = ml_dtypes.bfloat16

_cache = {}


def _preprocess(edge_index):
    """Partition/sort edges, build per-core idx / indicator arrays."""
    ei = np.asarray(edge_index)
    src = np.concatenate([ei[0], np.arange(N_NODES, dtype=np.int64)])
    dst = np.concatenate([ei[1], np.arange(N_NODES, dtype=np.int64)])
    deg = np.bincount(dst, minlength=N_NODES).astype(np.float64)
    dinv = (1.0 / np.sqrt(deg)).astype(np.float32)
    sqdeg = np.sqrt(deg).astype(np.float32)

    # self loops are handled by a dedicated per-block stream; drop them here
    e_src, e_dst = ei[0], ei[1]
    core = e_dst // RPC
    rel = e_dst - core * RPC
    blk = rel >> 7
    bucket = (e_src >= SPLIT).astype(np.int64)

    # group edges by (core, block, bucket); keep the (random) input order
    # within each group — sorted-by-src gathers alias HBM channels and run
    # ~3x slower than randomly ordered ones (measured).
    key = (core * NBLK + blk) * 2 + bucket
    order = np.argsort(key, kind="stable")
    g_src = e_src[order]
    g_rel = rel[order]
    gkey = (core * NBLK + blk) * 2 + bucket
    counts = np.bincount(gkey, minlength=NCORES * NBLK * 2)
    counts = counts.reshape(NCORES, NBLK, 2)

    # shared (cross-core) tile counts per (block, bucket)
    T = np.maximum(cdiv_arr(counts.max(axis=0), 128), 1)   # [NBLK, 2]
    T_lo, T_hi = T[:, 0], T[:, 1]

    # per-core padded streams
    starts = np.zeros(NCORES * NBLK * 2 + 1, np.int64)
    np.cumsum(counts.reshape(-1), out=starts[1:])

    per_core = []
    for c in range(NCORES):
        streams = {}
        for b_idx, tcounts in (("lo", T_lo), ("hi", T_hi)):
            k = 0 if b_idx == "lo" else 1
            idx_list = []
            rel_list = []
            for b in range(NBLK):
                gi = (c * NBLK + b) * 2 + k
                s, e = starts[gi], starts[gi + 1]
                n = e - s
                cap = tcounts[b] * 128
                assert n <= cap
                isrc = g_src[s:e]
                if k == 1:
                    isrc = isrc - SPLIT
                pad = cap - n
                idx_list.append(np.concatenate([isrc, np.zeros(pad, np.int64)]))
                rel_list.append(
                    np.concatenate([g_rel[s:e] - b * 128, np.full(pad, -1, np.int64)])
                )
            streams[b_idx] = (np.concatenate(idx_list), np.concatenate(rel_list))
        # self-loop stream: one tile per block, idx relative to own shard
        sidx = np.arange(RPC_PAD, dtype=np.int64)
        srel = np.where(sidx < RPC, sidx % 128, -1)
        sidx = np.where(sidx < RPC, sidx, 0)
        streams["self"] = (sidx, srel)
        per_core.append(streams)

    return dinv, sqdeg, T_lo, T_hi, per_core


def cdiv_arr(a, b):
    return -(-a // b)


def _chunk_list(n_tiles):
    """Split a stream of n_tiles into chunks of <= CHUNK_TILES."""
    out = []
    t = 0
    while t < n_tiles:
        ct = min(CHUNK_TILES, n_tiles - t)
        out.append((t, ct))
        t += ct
    return out


def _wrap_idx(idx_stream, chunks):
    """int16 idx array [128, sum(8*ct)], each chunk 16-wrapped separately."""
    cols = []
    for (t0, ct) in chunks:
        seg = idx_stream[t0 * 128:(t0 + ct) * 128].astype(np.int16)
        w = seg.reshape(-1, 16).T.copy()          # [16, ct*8]
        cols.append(np.tile(w, (8, 1)))           # [128, ct*8]
    return np.concatenate(cols, axis=1)


def _ind_tiles(rel_stream):
    """fp8 indicator array [128, T*128]; tile t cols t*128:(t+1)*128."""
    T = len(rel_stream) // 128
    on = (rel_stream[:, None] == np.arange(128)[None, :])
    on = on.reshape(T, 128, 128).transpose(1, 0, 2).reshape(128, T * 128)
    return on.astype(F8)


def _build_program(T_lo, T_hi, sim_single_core=False, ablate=None,
                   zero_bias=False):
    """ablate: None | 'noag' (skip collectives) | 'gathers' (only gathers+ind
    DMAs) | 'mm' (everything except gathers/ind DMAs/collectives)."""
    TL = int(T_lo.sum())
    TH = int(T_hi.sum())
    lo_chunks = _chunk_list(TL)
    hi_chunks = _chunk_list(TH)
    self_chunks = _chunk_list(NBLK)
    idx_cols = 8 * (TL + TH + NBLK)
    if sim_single_core:
        ablate = "noag"
    no_collectives = ablate is not None
    do_mm = ablate in (None, "noag", "mm")
    do_gather = ablate in (None, "noag", "gathers")

    nc = bacc.Bacc("TRN2", target_bir_lowering=False, debug=False,
                   num_devices=1 if sim_single_core else NCORES,
                   num_swdge_queues=NQUEUES)

    dt = mybir.dt
    xT_d = nc.dram_tensor("xT", [CH, RPC_PAD], dt.float32, kind="ExternalInput")
    w1_d = nc.dram_tensor("w1", [CH, CH], dt.float32, kind="ExternalInput")
    w2_d = nc.dram_tensor("w2", [CH, CH], dt.float32, kind="ExternalInput")
    wt_d = nc.dram_tensor("wt", [2, CH], dt.float32, kind="ExternalInput")
    wf_d = nc.dram_tensor("wf", [1, 2 * CH], dt.float32, kind="ExternalInput")
    b1_d = nc.dram_tensor("b1", [1, CH], dt.float32, kind="ExternalInput")
    b2_d = nc.dram_tensor("b2", [1, CH], dt.float32, kind="ExternalInput")
    bt_d = nc.dram_tensor("bt", [1, CH], dt.float32, kind="ExternalInput")
    bf_d = nc.dram_tensor("bf", [1, 1], dt.float32, kind="ExternalInput")
    tfT_d = nc.dram_tensor("tfT", [2, RPC_PAD], dt.float32, kind="ExternalInput")
    dinv_d = nc.dram_tensor("dinvc", [128, NBLK], dt.float32, kind="ExternalInput")
    sqdeg_d = nc.dram_tensor("sqdegc", [1, RPC_PAD], dt.float32, kind="ExternalInput")
    ident_d = nc.dram_tensor("ident", [128, 128], dt.float32, kind="ExternalInput")
    idx_d = nc.dram_tensor("idx", [128, idx_cols], dt.int16, kind="ExternalInput")
    indlo_d = nc.dram_tensor("indlo", [128, TL * 128], dt.float8e4, kind="ExternalInput")
    indhi_d = nc.dram_tensor("indhi", [128, TH * 128], dt.float8e4, kind="ExternalInput")
    indself_d = nc.dram_tensor("indself", [128, NBLK * 128], dt.float8e4, kind="ExternalInput")
    out_d = nc.dram_tensor("out", [128, NBLK], dt.float32, kind="ExternalOutput")

    rows_of = lambda b: min(128, RPC - b * 128)

    with tile.TileContext(nc) as tc:
        with tc.tile_pool(name="sbuf", bufs=1) as sb, \
             tc.tile_pool(name="psum", bufs=1, space="PSUM") as ps, \
             tc.tile_pool(name="dram", bufs=1, space="DRAM") as dr:
            nc.gpsimd.load_library(library_config.mlp)

            # ---- resident constants ----
            idx_sb = sb.tile([128, idx_cols], dt.int16)
            nc.sync.dma_start(idx_sb[:], idx_d[:])
            indself_sb = sb.tile([128, NBLK * 128], dt.float8e4)
            nc.sync.dma_start(indself_sb[:], indself_d[:])
            w1_sb = sb.tile([128, 2, CH], dt.float32)
            w2_sb = sb.tile([128, 2, CH], dt.float32)
            for k in range(2):
                nc.sync.dma_start(w1_sb[:, k, :], w1_d[k * 128:(k + 1) * 128, :])
                nc.sync.dma_start(w2_sb[:, k, :], w2_d[k * 128:(k + 1) * 128, :])
            wt_sb = sb.tile([2, CH], dt.float32)
            nc.sync.dma_start(wt_sb[:], wt_d[:])
            wf_sb = sb.tile([128, 2 * CH], dt.float32)
            nc.sync.dma_start(wf_sb[:], wf_d[:].to_broadcast([128, 2 * CH]))
            b1_sb = sb.tile([1, CH], dt.float32)
            nc.sync.dma_start(b1_sb[:], b1_d[:])
            b2_sb = sb.tile([1, CH], dt.float32)
            nc.sync.dma_start(b2_sb[:], b2_d[:])
            bt_sb = sb.tile([1, CH], dt.float32)
            nc.sync.dma_start(bt_sb[:], bt_d[:])
            bf_sb = sb.tile([128, 1], dt.float32)
            nc.sync.dma_start(bf_sb[:], bf_d[:].to_broadcast([128, 1]))
            dinv_sb = sb.tile([128, NBLK], dt.float32)
            nc.sync.dma_start(dinv_sb[:], dinv_d[:])
            ident = sb.tile([128, 128], dt.float32)
            nc.sync.dma_start(ident[:], ident_d[:])
            ones_row = sb.tile([1, 128], dt.float32)
            nc.vector.memset(ones_row[:], 1.0)

            def sqdeg_row(b, who):
                sq = sb.tile([1, 128], dt.float32, tag="sqrow", bufs=3,
                             name=f"sq{who}_{b}")
                nc.sync.dma_start(sq[:], sqdeg_d[:, b * 128:(b + 1) * 128])
                return sq

            # ---- AllGather buffers ----
            ag_in = [dr.tile([RPC, CH], dt.bfloat16, name=f"ag{l}_in", tag=f"ag{l}_in")
                     for l in range(2)]
            if no_collectives:
                # no collectives; gather from dummy DRAM tables
                ag_out = [dr.tile([N_NODES, CH], dt.bfloat16,
                                  name=f"tbl{l}", tag=f"tbl{l}") for l in range(2)]
            else:
                ag_out = [dr.tile([N_NODES, CH], dt.bfloat16, addr_space="Shared",
                                  name=f"ag{l}_out", tag=f"ag{l}_out") for l in range(2)]

            # ---- stage 1: xws1 = dinv * (x @ W1), bf16 ----
            def xw_block(b, lhsT_tiles, w_sb, ag_in_t, layer):
                """lhsT_tiles: callable k -> sbuf tile [128ch, 128nodes] f32."""
                rows = rows_of(b)
                xwp = ps.tile([128, CH], dt.float32, space="PSUM",
                              tag="xwp", bufs=2, name=f"xwp{layer}_{b}")
                for k in range(2):
                    nc.tensor.matmul(xwp[:], lhsT=lhsT_tiles(k), rhs=w_sb[:, k, :],
                                     start=(k == 0), stop=(k == 1))
                xws = sb.tile([128, CH], dt.bfloat16, tag="xws", bufs=3,
                              name=f"xws{layer}_{b}")
                nc.scalar.activation(xws[:], xwp[:],
                                     mybir.ActivationFunctionType.Copy,
                                     scale=dinv_sb[:, b:b + 1])
                nc.sync.dma_start(ag_in_t[b * 128:b * 128 + rows, :], xws[:rows, :])

            if do_mm:
                for b in range(NBLK):
                    def xT_tiles(k, b=b):
                        xT = sb.tile([128, 128], dt.float32, tag="xT", bufs=4,
                                     name=f"xT0_{b}_{k}")
                        nc.sync.dma_start(
                            xT[:], xT_d[k * 128:(k + 1) * 128,
                                        b * 128:(b + 1) * 128])
                        return xT
                    xw_block(b, xT_tiles, w1_sb, ag_in[0], 0)
                # x_time precompute: independent of the AllGathers; fills the
                # AG1 bubble.  xtime tiles stay resident for the head.
                xtime = []
                for b in range(NBLK):
                    tfb = sb.tile([2, 8 * 128], dt.float32, tag="tfb", bufs=2,
                                  name=f"tfb_{b // 8}") if b % 8 == 0 else tfb
                    if b % 8 == 0:
                        c0 = b * 128
                        cw = min(8 * 128, RPC_PAD - c0)
                        nc.sync.dma_start(tfb[:, :cw], tfT_d[:, c0:c0 + cw])
                    pt = ps.tile([128, CH], dt.float32, space="PSUM", tag="tp",
                                 bufs=2, name=f"pt_{b}")
                    nc.tensor.matmul(pt[:], lhsT=tfb[:, (b % 8) * 128:(b % 8 + 1) * 128],
                                     rhs=wt_sb[:], start=True,
                                     stop=zero_bias)
                    if not zero_bias:
                        nc.tensor.matmul(pt[:], lhsT=ones_row[:], rhs=bt_sb[:],
                                         start=False, stop=True)
                    xt_sb = sb.tile([128, CH], dt.bfloat16, tag="xtime", bufs=NBLK,
                                    name=f"xtime_{b}")
                    nc.scalar.activation(xt_sb[:], pt[:],
                                         mybir.ActivationFunctionType.Relu)
                    xtime.append(xt_sb)
            if not no_collectives:
                nc.gpsimd.collective_compute(
                    "AllGather", mybir.AluOpType.bypass,
                    replica_groups=[list(range(NCORES))],
                    ins=[ag_in[0][:]], outs=[ag_out[0][:]])

            # ---- scatter stage helper ----
            qctr = [0]

            class Stream:
                """Lazy chunked gather stream: tile index -> (rhs, ind) APs."""

                def __init__(self, name, sfx, chunks, colbase, src_ap, ind_dram,
                             ind_sb, gtag, itag, bufs):
                    self.name, self.sfx = name, sfx
                    self.chunks, self.colbase = chunks, colbase
                    self.src_ap, self.ind_dram, self.ind_sb = src_ap, ind_dram, ind_sb
                    self.gtag, self.itag, self.bufs = gtag, itag, bufs
                    self.ci = -1
                    self.g = None
                    self.ind = None
                    self.cursor = 0

                def _fetch(self, ci):
                    t0, ct = self.chunks[ci]
                    g = sb.tile([128, ct, CH], dt.bfloat16, tag=self.gtag,
                                bufs=self.bufs, name=f"g{self.name}{self.sfx}_{ci}")
                    nidx = ct * 128
                    colb = self.colbase + 8 * t0
                    if do_gather:
                        nc.gpsimd.dma_gather(g[:], self.src_ap,
                                             idx_sb[:, colb:colb + 8 * ct],
                                             nidx, nidx, CH,
                                             queue_num=qctr[0] % NQUEUES)
                        qctr[0] += 1
                    else:
                        nc.gpsimd.memset(g[:, 0, 0:1], 0)
                    self.g = g
                    if self.ind_dram is not None:
                        ind = sb.tile([128, ct * 128], dt.float8e4, tag=self.itag,
                                      bufs=self.bufs,
                                      name=f"i{self.name}{self.sfx}_{ci}")
                        if do_gather:
                            # ACT HWDGE ring: keeps indicator loads off the SP
                            # ring used by xws/xT traffic
                            nc.scalar.dma_start(
                                ind[:], self.ind_dram[:, t0 * 128:(t0 + ct) * 128])
                        else:
                            nc.gpsimd.memset(ind[:, 0:1], 0)
                        self.ind = ind
                    self.ci = ci
                    self.t0 = t0

                def next_tile(self):
                    cur = self.cursor
                    ci = cur // CHUNK_TILES
                    if ci != self.ci:
                        self._fetch(ci)
                    slot = cur - self.t0
                    self.cursor = cur + 1
                    rhs = self.g[:, slot, :]
                    if self.ind_dram is not None:
                        ind = self.ind[:, slot * 128:(slot + 1) * 128]
                    else:
                        ind = self.ind_sb[:, cur * 128:(cur + 1) * 128]
                    return rhs, ind

            def scatter_layer(layer, table, own_shard, b_row, post_block):
                """Aggregate: for each block, psum = sqdeg(x)b + sum ind.T @ gathered.
                post_block(b, agg_psum) consumes the accumulated psum."""
                sfx = f"L{layer}"
                lo = Stream("lo", sfx, lo_chunks, 0, table[:SPLIT, :],
                            indlo_d, None, "glo", "ilo", 4)
                hi = Stream("hi", sfx, hi_chunks, 8 * TL, table[SPLIT:, :],
                            indhi_d, None, "ghi", "ihi", 4)
                # self stream: own-shard rows in order -> plain strided DMA
                # instead of dma_gather (sequential read, no SWDGE cost)
                gself = sb.tile([128, NBLK, CH], dt.bfloat16, tag="gself", bufs=1,
                                name=f"gself{sfx}")
                if do_gather:
                    nc.vector.memset(gself[:, NBLK - 1, :], 0.0)
                    full = (NBLK - 1) * 128      # 6144 full rows
                    nc.sync.dma_start(
                        gself[:, :NBLK - 1, :],
                        own_shard[:full, :].rearrange("(b p) c -> p b c", p=128))
                    last = RPC - full            # 106
                    nc.sync.dma_start(gself[:last, NBLK - 1, :],
                                      own_shard[full:RPC, :])
                else:
                    nc.gpsimd.memset(gself[:, 0, 0:1], 0)

                # software pipeline: run block b's post work while block b+1's
                # aggregation matmuls keep PE busy (the post chain waits on an
                # ACT epilogue of the psum)
                pending = []
                for b in range(NBLK):
                    first = [True]
                    if do_mm:
                        agg = ps.tile([128, CH], dt.float32, space="PSUM",
                                      tag="agg", bufs=4, name=f"agg{sfx}_{b}")
                        if not zero_bias:
                            # rank-1 bias: psum = sqdeg (x) b
                            sq = sqdeg_row(b, sfx)
                            nc.tensor.matmul(agg[:], lhsT=sq[:],
                                             rhs=b_row[:], start=True, stop=False)
                            first[0] = False

                    def mm(ind, rhs, stop=False):
                        nc.tensor.matmul(agg[:], lhsT=ind, rhs=rhs,
                                         start=first[0], stop=stop)
                        first[0] = False

                    for _ in range(int(T_lo[b])):
                        rhs, ind = lo.next_tile()
                        if do_mm:
                            mm(ind, rhs)
                    for _ in range(int(T_hi[b])):
                        rhs, ind = hi.next_tile()
                        if do_mm:
                            mm(ind, rhs)
                    # self tile (identity indicator, from the own-shard copy)
                    if do_mm:
                        mm(indself_sb[:, b * 128:(b + 1) * 128], gself[:, b, :],
                           stop=True)
                        post_block(b, agg)

            # ---- stage 2: layer-1 aggregation + h @ W2 ----
            h_hold = {}

            def post1(b, agg):
                h_sb = sb.tile([128, CH], dt.float32, tag="h", bufs=2, name=f"h_{b}")
                nc.scalar.activation(h_sb[:], agg[:],
                                     mybir.ActivationFunctionType.Relu,
                                     scale=dinv_sb[:, b:b + 1])
                h_hold[b] = h_sb

            def h_tiles(b):
                return h_hold.pop(b)

            # interleave: aggregate block b then immediately do its h @ W2
            def post1_and_xw2(b, agg):
                post1(b, agg)
                x_sb = h_tiles(b)

                def hT_tiles(k, b=b, x_sb=x_sb):
                    tp = ps.tile([128, 128], dt.float32, space="PSUM",
                                 tag="tp", bufs=2, name=f"tp1_{b}_{k}")
                    nc.tensor.transpose(tp[:], x_sb[:, k * 128:(k + 1) * 128],
                                        ident[:])
                    xT = sb.tile([128, 128], dt.float32, tag="xT", bufs=4,
                                 name=f"xT1_{b}_{k}")
                    nc.vector.tensor_copy(xT[:], tp[:])
                    return xT

                xw_block(b, hT_tiles, w2_sb, ag_in[1], 1)

            scatter_layer(0, ag_out[0], ag_in[0], b1_sb, post1_and_xw2)
            if not no_collectives:
                nc.gpsimd.collective_compute(
                    "AllGather", mybir.AluOpType.bypass,
                    replica_groups=[list(range(NCORES))],
                    ins=[ag_in[1][:]], outs=[ag_out[1][:]])

            # ---- stage 3: layer-2 aggregation + head ----
            dot_acc = sb.tile([128, NBLK], dt.float32, tag="dot_acc", bufs=1)

            def post2(b, agg):
                comb = sb.tile([128, 2 * CH], dt.float32, tag="comb", bufs=2,
                               name=f"comb_{b}")
                nc.scalar.activation(comb[:, :CH], agg[:],
                                     mybir.ActivationFunctionType.Copy,
                                     scale=dinv_sb[:, b:b + 1])
                # x_time was precomputed during stage 1 (bf16)
                nc.vector.tensor_copy(comb[:, CH:], xtime[b][:])
                prod = sb.tile([128, 2 * CH], dt.float32, tag="prod", bufs=2,
                               name=f"prod_{b}")
                nc.vector.tensor_tensor(out=prod[:], in0=comb[:], in1=wf_sb[:],
                                        op=mybir.AluOpType.mult)
                if zero_bias:
                    nc.vector.tensor_reduce(out=dot_acc[:, b:b + 1], in_=prod[:],
                                            op=mybir.AluOpType.add,
                                            axis=mybir.AxisListType.X)
                else:
                    dcol = sb.tile([128, 1], dt.float32, tag="dcol", bufs=2,
                                   name=f"dcol_{b}")
                    nc.vector.tensor_reduce(out=dcol[:], in_=prod[:],
                                            op=mybir.AluOpType.add,
                                            axis=mybir.AxisListType.X)
                    nc.vector.tensor_tensor(out=dot_acc[:, b:b + 1], in0=dcol[:],
                                            in1=bf_sb[:], op=mybir.AluOpType.add)

            scatter_layer(1, ag_out[1], ag_in[1], b2_sb, post2)

            if not do_mm:
                nc.vector.memset(dot_acc[:], 0.0)
            nc.sync.dma_start(out_d[:], dot_acc[:])

    nc.compile()
    return nc


def _host_inputs(inputs, dinv, sqdeg, T_lo, T_hi, per_core):
    x = np.asarray(inputs["x"], np.float32)
    tf = np.asarray(inputs["time_features"], np.float32)
    W1 = np.asarray(inputs["W1"], np.float32)
    W2 = np.asarray(inputs["W2"], np.float32)
    Wt = np.asarray(inputs["Wt"], np.float32)
    Wf = np.asarray(inputs["Wf"], np.float32).reshape(-1)
    b1 = np.asarray(inputs["b1"], np.float32)
    b2 = np.asarray(inputs["b2"], np.float32)
    bt = np.asarray(inputs["bt"], np.float32)
    bf = np.asarray(inputs["bf"], np.float32).reshape(1, 1)

    TL = int(T_lo.sum())
    TH = int(T_hi.sum())
    lo_chunks = _chunk_list(TL)
    hi_chunks = _chunk_list(TH)

    in_maps = []
    for c in range(NCORES):
        st = per_core[c]
        idx = np.concatenate([
            _wrap_idx(st["lo"][0], lo_chunks),
            _wrap_idx(st["hi"][0], hi_chunks),
            _wrap_idx(st["self"][0], _chunk_list(NBLK)),
        ], axis=1)
        r0 = c * RPC
        tfT = np.zeros((2, RPC_PAD), np.float32)
        tfT[:, :RPC] = tf[r0:r0 + RPC].T
        dv = np.zeros(RPC_PAD, np.float32)
        dv[:RPC] = dinv[r0:r0 + RPC]
        dinv_c = dv.reshape(NBLK, 128).T.copy()
        sq_c = np.zeros((1, RPC_PAD), np.float32)
        sq_c[0, :RPC] = sqdeg[r0:r0 + RPC]
        xT = np.zeros((CH, RPC_PAD), np.float32)
        xT[:, :RPC] = x[r0:r0 + RPC].T
        in_maps.append({
            "xT": xT,
            "w1": W1, "w2": W2, "wt": Wt,
            "wf": Wf[None, :].copy(),
            "b1": b1[None, :], "b2": b2[None, :], "bt": bt[None, :], "bf": bf,
            "tfT": tfT,
            "dinvc": dinv_c,
            "sqdegc": sq_c,
            "ident": np.eye(128, dtype=np.float32),
            "idx": idx,
            "indlo": _ind_tiles(st["lo"][1]),
            "indhi": _ind_tiles(st["hi"][1]),
            "indself": _ind_tiles(st["self"][1]),
        })
    return in_maps


def _run(inputs, trace=False):
    dinv, sqdeg, T_lo, T_hi, per_core = _preprocess(inputs["edge_index"])
    zero_bias = not any(np.any(np.asarray(inputs[k])) for k in ("b1", "b2", "bt", "bf"))
    key = (tuple(T_lo.tolist()), tuple(T_hi.tolist()), zero_bias)
    if key not in _cache:
        _cache.clear()
        _cache[key] = _build_program(T_lo, T_hi, zero_bias=zero_bias)
    nc = _cache[key]
    in_maps = _host_inputs(inputs, dinv, sqdeg, T_lo, T_hi, per_core)
    res = run_bass_kernel_spmd(nc, in_maps, core_ids=list(range(NCORES)),
                               trace=trace)
    # out is [128, NBLK] with element (p, b) = row b*128+p of the shard
    out = np.concatenate(
        [np.asarray(res.results[c]["out"]).T.reshape(-1, 1)[:RPC]
         for c in range(NCORES)], axis=0)
    return out.astype(np.float32), res


def kernel(**inputs):
    out, _ = _run(inputs, trace=False)
    return out

